# revision 9
# baseline (speedup 1.0000x reference)
"""Depthwise causal conv1d kernel for Trainium2 (8 NeuronCores, SPMD).

Problem: x [B=8, T=4096, C=512] f32, weight [C=512, K=4] f32.
out[b, t, c] = sum_k weight[c, k] * x[b, t - 3 + k, c]   (causal, zero-pad)

Strategy (v6):
  - Data-parallel over batch: core b handles x[b].
  - Host-side layout: channels-first x[b].T padded with 3 leading zeros
    along time -> [C=512, T+3=4099] fp16, as [128, 4*4099] (4 channel
    chunks of 128 on partitions). fp16 halves HBM traffic; accumulation
    stays fp32 in PSUM.
  - PE computes taps 0..2 as accumulating diag-matmuls (96 x 512-col
    matmuls, 216ns each at full clock); DVE fuses tap 3 + PSUM drain +
    fp16 cast in ONE scalar_tensor_tensor per piece:
    out = (x_shift3 * w3) + psum.
  - The DVE merge train is the pipeline's tail, so it starts as early
    as possible (the first merge piece is only 512 cols, right after
    chunk 0's first j-tile) and ends small (the last half is merged
    and shipped as 1024+512+512-col pieces).
  - The 12 diag stationary tiles are pre-built ON HOST; chunk 0's
    three tiles ship as a separate tiny DMA armed first, so PE's
    first matmul gates only on ~0.1MB of transfers.
  - 9 x 512-col dummy matmuls on a GpSimd-memset tile hold the PE
    clock at speed from ~7us until real data lands (~10.5us).
  - Sequencer roles strictly separated: SP arms all inputs; ACT arms
    outputs (descriptors wait in the rings on merge semaphores and
    burst when they fire); DVE only merges; PE only matmuls. The very
    last output piece is armed on SP so the two final bursts drain
    through different queues in parallel.
"""

import numpy as np

B, T, C, K = 8, 4096, 512, 4
P = 128  # partitions
NCHUNK = C // P  # 4 channel chunks
TJ = 512  # time-tile (free dim) per matmul; one PSUM bank
NJ = T // TJ  # 8 j-tiles per chunk
TP = T + K - 1  # padded time = 4099
TH = T // 2  # half-chunk = 2048 cols
THP = TH + K - 1  # half tile incl halo = 2051
NW = 3 * NCHUNK + 1  # 12 diag stationaries (taps 0..2) + c3-tap3 for the tail

_compiled = None


def _build():
    import concourse.bacc as bacc
    import concourse.mybir as mybir
    from concourse.tile import TileContext

    f32 = mybir.dt.float32
    f16 = mybir.dt.float16
    nc = bacc.Bacc(enable_partition_id=False)

    wdiag_d = nc.declare_dram_parameter("wdiag", [P, NW * P], f16, isOutput=False)
    wcol_d = nc.declare_dram_parameter("wt", [P, NCHUNK * K], f32, isOutput=False)
    xw_d = nc.declare_dram_parameter("xw", [P, NCHUNK * TP], f16, isOutput=False)
    out_d = nc.declare_dram_parameter("out", [C, T], f16, isOutput=True)

    with TileContext(nc) as tc:
        with (
            tc.tile_pool(name="xpool", bufs=1) as xpool,
            tc.tile_pool(name="wpool", bufs=1) as wpool,
            tc.tile_pool(name="opool", bufs=4) as opool,
            tc.tile_pool(name="ppool", bufs=2, space="PSUM") as ppool,
        ):
            wdiag = wpool.tile([P, NW * P], f16, tag="wdiag")
            wcol = wpool.tile([P, NCHUNK * K], f32, tag="wcol")
            warm = wpool.tile([P, TJ], f16, tag="warm")
            # chunk 0 split into two tiles so the first-half matmuls gate
            # only on the first transfers; chunks 1-3 arrive early enough
            # as single tiles
            xt0 = [
                xpool.tile([P, THP], f16, name=f"xt0{h}", tag=f"xt0{h}")
                for h in range(2)
            ]
            xts = [
                xpool.tile([P, TP], f16, name=f"xt{c}", tag=f"xt{c}")
                for c in range(1, NCHUNK)
            ]

            # --- SP arms all inputs; ring descriptors execute in arm
            # order, so this order IS the arrival order. PE's first j-tile
            # needs only wdiag[c0] (0.1MB) + x cols [0:515].
            nc.sync.dma_start(out=wdiag[:, : 3 * P], in_=wdiag_d[:, : 3 * P])
            nc.sync.dma_start(out=xt0[0], in_=xw_d[:, :THP])
            nc.sync.dma_start(out=wcol, in_=wcol_d[:, :])
            nc.sync.dma_start(out=wdiag[:, 3 * P :], in_=wdiag_d[:, 3 * P :])
            nc.sync.dma_start(out=xt0[1], in_=xw_d[:, TH : TH + THP])
            for c in range(1, NCHUNK):
                nc.sync.dma_start(
                    out=xts[c - 1], in_=xw_d[:, c * TP : (c + 1) * TP]
                )

            # --- PE clock pre-warm: dummy matmuls on a GpSimd-memset tile
            # hold the pstate ramp until real data lands
            nc.gpsimd.memset(warm, 0)
            pwarm = ppool.tile([P, TH], f32, name="pw", tag="pt")
            for i in range(9):
                nc.tensor.matmul(
                    pwarm[:, :TJ], warm[:, :P], warm, start=True, stop=True
                )

            # --- main loop: PE 3 taps -> PSUM; DVE fuses tap3 + drain.
            # Merge pieces sized to start the DVE train early (512-col
            # first piece) and end it small (1024/512/512 tail).
            for c in range(NCHUNK):
                w3 = wcol[:, c * K + 3 : c * K + 4]
                ot = opool.tile([P, T], f16, tag="ot")
                for half in range(2):
                    xv = xt0[half] if c == 0 else xts[c - 1]
                    vbase = 0 if c == 0 else half * TH
                    pt = ppool.tile([P, TH], f32, name="pt", tag="pt")
                    tail = c == NCHUNK - 1 and half == 1
                    for j4 in range(NJ // 2):
                        # final j-tile: 4 taps so ACT can drain it by copy
                        ntap = 4 if tail and j4 == NJ // 2 - 1 else 3
                        for k in range(ntap):
                            lo = vbase + j4 * TJ + k
                            wi = 12 if k == 3 else 3 * c + k
                            nc.tensor.matmul(
                                pt[:, j4 * TJ : (j4 + 1) * TJ],
                                wdiag[:, wi * P : (wi + 1) * P],
                                xv[:, lo : lo + TJ],
                                start=(k == 0),
                                stop=(k == ntap - 1),
                            )
                    hbase = half * TH
                    if c == 0 and half == 0:
                        pieces = [TJ, TH - TJ]  # start the train early
                    elif tail:
                        pieces = [3 * TJ]  # DVE side; ACT copies the rest
                    else:
                        pieces = [TH]
                    plo = 0
                    for sz in pieces:
                        vlo = vbase + plo + 3
                        olo = hbase + plo
                        nc.vector.scalar_tensor_tensor(
                            out=ot[:, olo : olo + sz],
                            in0=xv[:, vlo : vlo + sz],
                            scalar=w3,
                            in1=pt[:, plo : plo + sz],
                            op0=mybir.AluOpType.mult,
                            op1=mybir.AluOpType.add,
                        )
                        nc.scalar.dma_start(
                            out=out_d[c * P : (c + 1) * P, olo : olo + sz],
                            in_=ot[:, olo : olo + sz],
                        )
                        plo += sz
                    if tail:
                        # ACT drains the last 512-col slice (tap3 already in
                        # PSUM) in parallel with DVE's piece; its dma goes
                        # via SP so the two final bursts use parallel queues
                        olo = hbase + 3 * TJ
                        nc.scalar.copy(ot[:, olo : olo + TJ], pt[:, 3 * TJ :])
                        nc.sync.dma_start(
                            out=out_d[c * P : (c + 1) * P, olo : olo + TJ],
                            in_=ot[:, olo : olo + TJ],
                        )

    nc.compile()
    return nc


def _prep_inputs(x: np.ndarray, weight: np.ndarray):
    # wcol[p, chunk*K + k] = weight[chunk*P + p, k]
    wcol = np.ascontiguousarray(
        weight.reshape(NCHUNK, P, K).transpose(1, 0, 2).reshape(P, NCHUNK * K)
    ).astype(np.float32)
    # wdiag[p, (3c+k)*P + m] = weight[c*P+p, k] * (m == p): diag stationaries
    wdiag = np.zeros((P, NW * P), dtype=np.float16)
    rng = np.arange(P)
    for c in range(NCHUNK):
        for k in range(3):
            wdiag[rng, (3 * c + k) * P + rng] = weight[c * P + rng, k].astype(
                np.float16
            )
    # c3-tap3 tile: lets PE finish the very last j-tile with all 4 taps so
    # ACT can drain that slice with a pure copy, in parallel with DVE
    wdiag[rng, 12 * P + rng] = weight[3 * P + rng, 3].astype(np.float16)
    xs = []
    for b in range(B):
        xp = np.zeros((C, TP), dtype=np.float32)
        xp[:, K - 1 :] = x[b].T  # [512, 4099], 3 leading zeros
        xw = np.ascontiguousarray(
            xp.reshape(NCHUNK, P, TP).transpose(1, 0, 2).reshape(P, NCHUNK * TP)
        ).astype(np.float16)
        xs.append(xw)
    return xs, wcol, wdiag


def _in_maps(x: np.ndarray, weight: np.ndarray):
    xs, wcol, wdiag = _prep_inputs(x, weight)
    return [{"xw": xs[b], "wt": wcol, "wdiag": wdiag} for b in range(B)]


def _ensure_axon_hooks():
    """This image's antenv package lacks axon_hooks; synthesize it so a
    trace=True / BASS_TRACE run of run_bass_kernel_spmd can profile
    instead of crashing on import."""
    import sys
    import types

    if "antenv.axon_hooks" in sys.modules:
        return
    mod = types.ModuleType("antenv.axon_hooks")
    state = {"hook": None}
    mod.set_axon_ntff_profile_hook = lambda h: state.__setitem__("hook", h)
    mod.get_axon_ntff_profile_hook = lambda: state["hook"]
    sys.modules["antenv.axon_hooks"] = mod
    try:
        if "/root/.axon_site" not in sys.path:
            sys.path.insert(0, "/root/.axon_site")
        from trn_agent_boot.trn_boot import _ntff_profile_via_ctypes

        mod.set_axon_ntff_profile_hook(
            _ntff_profile_via_ctypes("/opt/axon/libaxon_pjrt.so")
        )
    except Exception:
        pass  # hook stays None; concourse degrades to no-trace


def kernel(x: np.ndarray, weight: np.ndarray) -> np.ndarray:
    global _compiled
    _ensure_axon_hooks()
    from concourse import bass_utils

    x = np.ascontiguousarray(x, dtype=np.float32)
    weight = np.ascontiguousarray(weight, dtype=np.float32)

    if _compiled is None:
        _compiled = _build()
    nc = _compiled

    in_maps = _in_maps(x, weight)
    res = bass_utils.run_bass_kernel_spmd(nc, in_maps, core_ids=list(range(B)))

    out = np.empty((B, T, C), dtype=np.float32)
    for b in range(B):
        out[b] = np.asarray(res.results[b]["out"]).astype(np.float32).T
    return out


# revision 10
# speedup vs baseline: 1.0314x; 1.0314x over previous
"""Depthwise causal conv1d kernel for Trainium2 (8 NeuronCores, SPMD).

Problem: x [B=8, T=4096, C=512] f32, weight [C=512, K=4] f32.
out[b, t, c] = sum_k weight[c, k] * x[b, t - 3 + k, c]   (causal, zero-pad)

Strategy (final):
  - Data-parallel over batch: core b handles x[b].
  - Host-side layout: channels-first x[b].T padded with 3 leading zeros
    along time -> [C=512, T+3=4099] fp16, as [128, 4*4099] (4 channel
    chunks of 128 on partitions). fp16 halves HBM traffic; accumulation
    stays fp32 in PSUM.
  - PE computes taps 0..2 as accumulating diag-matmuls (96 x 512-col
    matmuls, 216ns each at full clock); DVE fuses tap 3 + PSUM drain +
    fp16 cast in ONE scalar_tensor_tensor per piece:
    out = (x_shift3 * w3) + psum.
  - The DVE merge train is the pipeline's tail, so it starts as early
    as possible (the first merge piece is only 512 cols, right after
    chunk 0's first j-tile) and ends small (the last half is merged
    and shipped as 1024+512+512-col pieces).
  - The 12 diag stationary tiles are pre-built ON HOST; chunk 0's
    three tiles ship as a separate tiny DMA armed first, so PE's
    first matmul gates only on ~0.1MB of transfers.
  - 9 x 512-col dummy matmuls on a GpSimd-memset tile hold the PE
    clock at speed from ~7us until real data lands (~10.5us).
  - Sequencer roles strictly separated: SP arms all inputs; ACT arms
    outputs (descriptors wait in the rings on merge semaphores and
    burst when they fire); DVE only merges; PE only matmuls. The very
    last output piece is armed on SP so the two final bursts drain
    through different queues in parallel.
"""

import numpy as np

B, T, C, K = 8, 4096, 512, 4
P = 128  # partitions
NCHUNK = C // P  # 4 channel chunks
TJ = 512  # time-tile (free dim) per matmul; one PSUM bank
NJ = T // TJ  # 8 j-tiles per chunk
TP = T + K - 1  # padded time = 4099
TH = T // 2  # half-chunk = 2048 cols
THP = TH + K - 1  # half tile incl halo = 2051
NW = 3 * NCHUNK  # 12 host-built diag stationaries (taps 0..2)

_compiled = None


def _build():
    import concourse.bacc as bacc
    import concourse.mybir as mybir
    from concourse.tile import TileContext

    f32 = mybir.dt.float32
    f16 = mybir.dt.float16
    nc = bacc.Bacc(enable_partition_id=False)

    wdiag_d = nc.declare_dram_parameter("wdiag", [P, NW * P], f16, isOutput=False)
    wcol_d = nc.declare_dram_parameter("wt", [P, NCHUNK * K], f32, isOutput=False)
    xw_d = nc.declare_dram_parameter("xw", [P, NCHUNK * TP], f16, isOutput=False)
    out_d = nc.declare_dram_parameter("out", [C, T], f16, isOutput=True)

    with TileContext(nc) as tc:
        with (
            tc.tile_pool(name="xpool", bufs=1) as xpool,
            tc.tile_pool(name="wpool", bufs=1) as wpool,
            tc.tile_pool(name="opool", bufs=4) as opool,
            tc.tile_pool(name="ppool", bufs=2, space="PSUM") as ppool,
        ):
            wdiag = wpool.tile([P, NW * P], f16, tag="wdiag")
            wcol = wpool.tile([P, NCHUNK * K], f32, tag="wcol")
            warm = wpool.tile([P, TJ], f16, tag="warm")
            # chunk 0 split into two tiles so the first-half matmuls gate
            # only on the first transfers; chunks 1-3 arrive early enough
            # as single tiles
            xt0 = [
                xpool.tile([P, THP], f16, name=f"xt0{h}", tag=f"xt0{h}")
                for h in range(2)
            ]
            xts = [
                xpool.tile([P, TP], f16, name=f"xt{c}", tag=f"xt{c}")
                for c in range(1, NCHUNK)
            ]

            # --- SP arms all inputs; ring descriptors execute in arm
            # order, so this order IS the arrival order. PE's first j-tile
            # needs only wdiag[c0] (0.1MB) + x cols [0:515].
            nc.sync.dma_start(out=wdiag[:, : 3 * P], in_=wdiag_d[:, : 3 * P])
            nc.sync.dma_start(out=xt0[0], in_=xw_d[:, :THP])
            nc.sync.dma_start(out=wcol, in_=wcol_d[:, :])
            nc.sync.dma_start(out=wdiag[:, 3 * P :], in_=wdiag_d[:, 3 * P :])
            nc.sync.dma_start(out=xt0[1], in_=xw_d[:, TH : TH + THP])
            for c in range(1, NCHUNK):
                nc.sync.dma_start(
                    out=xts[c - 1], in_=xw_d[:, c * TP : (c + 1) * TP]
                )

            # --- PE clock pre-warm: dummy matmuls on a GpSimd-memset tile
            # hold the pstate ramp until real data lands
            nc.gpsimd.memset(warm, 0)
            pwarm = ppool.tile([P, TH], f32, name="pw", tag="pt")
            for i in range(9):
                nc.tensor.matmul(
                    pwarm[:, :TJ], warm[:, :P], warm, start=True, stop=True
                )

            # --- main loop: PE 3 taps -> PSUM; DVE fuses tap3 + drain.
            # Merge pieces sized to start the DVE train early (512-col
            # first piece) and end it small (1024/512/512 tail).
            for c in range(NCHUNK):
                w3 = wcol[:, c * K + 3 : c * K + 4]
                ot = opool.tile([P, T], f16, tag="ot")
                for half in range(2):
                    xv = xt0[half] if c == 0 else xts[c - 1]
                    vbase = 0 if c == 0 else half * TH
                    pt = ppool.tile([P, TH], f32, name="pt", tag="pt")
                    for j4 in range(NJ // 2):
                        for k in range(3):
                            lo = vbase + j4 * TJ + k
                            nc.tensor.matmul(
                                pt[:, j4 * TJ : (j4 + 1) * TJ],
                                wdiag[:, (3 * c + k) * P : (3 * c + k + 1) * P],
                                xv[:, lo : lo + TJ],
                                start=(k == 0),
                                stop=(k == 2),
                            )
                    hbase = half * TH
                    if c == 0 and half == 0:
                        pieces = [TJ, TH - TJ]  # start the train early
                    elif c == NCHUNK - 1 and half == 1:
                        pieces = [TH // 2, TH // 4, TH // 4]  # small tail
                    else:
                        pieces = [TH]
                    plo = 0
                    for pi, sz in enumerate(pieces):
                        vlo = vbase + plo + 3
                        olo = hbase + plo
                        nc.vector.scalar_tensor_tensor(
                            out=ot[:, olo : olo + sz],
                            in0=xv[:, vlo : vlo + sz],
                            scalar=w3,
                            in1=pt[:, plo : plo + sz],
                            op0=mybir.AluOpType.mult,
                            op1=mybir.AluOpType.add,
                        )
                        # outputs armed on ACT; the very last piece goes
                        # via SP so the two final bursts use different
                        # queues in parallel
                        last_piece = (
                            c == NCHUNK - 1 and half == 1 and pi == len(pieces) - 1
                        )
                        eng = nc.sync if last_piece else nc.scalar
                        eng.dma_start(
                            out=out_d[c * P : (c + 1) * P, olo : olo + sz],
                            in_=ot[:, olo : olo + sz],
                        )
                        plo += sz

    nc.compile()
    return nc


def _prep_inputs(x: np.ndarray, weight: np.ndarray):
    # wcol[p, chunk*K + k] = weight[chunk*P + p, k]
    wcol = np.ascontiguousarray(
        weight.reshape(NCHUNK, P, K).transpose(1, 0, 2).reshape(P, NCHUNK * K)
    ).astype(np.float32)
    # wdiag[p, (3c+k)*P + m] = weight[c*P+p, k] * (m == p): diag stationaries
    wdiag = np.zeros((P, NW * P), dtype=np.float16)
    rng = np.arange(P)
    for c in range(NCHUNK):
        for k in range(3):
            wdiag[rng, (3 * c + k) * P + rng] = weight[c * P + rng, k].astype(
                np.float16
            )
    xs = []
    for b in range(B):
        xp = np.zeros((C, TP), dtype=np.float32)
        xp[:, K - 1 :] = x[b].T  # [512, 4099], 3 leading zeros
        xw = np.ascontiguousarray(
            xp.reshape(NCHUNK, P, TP).transpose(1, 0, 2).reshape(P, NCHUNK * TP)
        ).astype(np.float16)
        xs.append(xw)
    return xs, wcol, wdiag


def _in_maps(x: np.ndarray, weight: np.ndarray):
    xs, wcol, wdiag = _prep_inputs(x, weight)
    return [{"xw": xs[b], "wt": wcol, "wdiag": wdiag} for b in range(B)]


def _ensure_axon_hooks():
    """This image's antenv package lacks axon_hooks; synthesize it so a
    trace=True / BASS_TRACE run of run_bass_kernel_spmd can profile
    instead of crashing on import."""
    import sys
    import types

    if "antenv.axon_hooks" in sys.modules:
        return
    mod = types.ModuleType("antenv.axon_hooks")
    state = {"hook": None}
    mod.set_axon_ntff_profile_hook = lambda h: state.__setitem__("hook", h)
    mod.get_axon_ntff_profile_hook = lambda: state["hook"]
    sys.modules["antenv.axon_hooks"] = mod
    try:
        if "/root/.axon_site" not in sys.path:
            sys.path.insert(0, "/root/.axon_site")
        from trn_agent_boot.trn_boot import _ntff_profile_via_ctypes

        mod.set_axon_ntff_profile_hook(
            _ntff_profile_via_ctypes("/opt/axon/libaxon_pjrt.so")
        )
    except Exception:
        pass  # hook stays None; concourse degrades to no-trace


def kernel(x: np.ndarray, weight: np.ndarray) -> np.ndarray:
    global _compiled
    _ensure_axon_hooks()
    from concourse import bass_utils

    x = np.ascontiguousarray(x, dtype=np.float32)
    weight = np.ascontiguousarray(weight, dtype=np.float32)

    if _compiled is None:
        _compiled = _build()
    nc = _compiled

    in_maps = _in_maps(x, weight)
    res = bass_utils.run_bass_kernel_spmd(nc, in_maps, core_ids=list(range(B)))

    out = np.empty((B, T, C), dtype=np.float32)
    for b in range(B):
        out[b] = np.asarray(res.results[b]["out"]).astype(np.float32).T
    return out
